# revision 25
# baseline (speedup 1.0000x reference)
"""Trainium2 Bass kernel for nn_BCIM_45861660787130 (pooling / box-filter sim).

Math per sample (C=128 channels, 32x32 spatial = S=1024 pixels):
  unit = p / ||p||_C
  wmean = 3x3 zero-padded box mean of unit (per channel)
  sim = <unit, wmean>_C          # per pixel
  out = p * sim, then channel deinterleave c=(f*2+e) -> [e*S + s, f]

Strategy (data-parallel over batch, 32 samples/core):
  - HOST pre-pass: transpose each sample to pixel-major [s128, k, c'] with
    channels permuted to c' = e*64+f, and cast to bf16.  The device then
    needs NO transposes and NO channel shuffle; input DMA is halved and has
    2KB/partition contiguous lines.
  - Per (sample b, chunk k of 128 pixels):
      ss[s]  = sum_c pin^2           (DVE tensor_tensor_reduce)
      nrm    = sqrt(ss)  (ACT),  rinv = 1/nrm  (DVE)
      u      = pin * rinv            (DVE tensor_scalar, bf16 4x)
  - Box filter = block-tridiagonal matmuls on PE (bf16, N=512, groups of 4
    samples):  box_k = Bd^T u_k + Bp^T u_{k-1} + Bn^T u_{k+1}  (PSUM acc)
  - evac = box PSUM -> SBUF bf16 (ACT Copy, batched FD=512)
  - zr[s] = sum_c pin*evac = sim*nrm (DVE TTR);  out = u*zr = p*sim exactly
  - DMA out per (sample, e): [128, 8, 64] -> DRAM [8, 128, 64].
"""

import os
import sys

sys.path.insert(0, "/opt/trn_rl_repo")

import numpy as np
import ml_dtypes

from concourse import bacc, bass, mybir, tile
from concourse.bass_utils import run_bass_kernel_spmd

F32 = mybir.dt.float32
BF16 = mybir.dt.bfloat16
AF = mybir.ActivationFunctionType
ALU = mybir.AluOpType
AX = mybir.AxisListType

B_PER_CORE = 32  # samples per core
NS = 4  # samples per group (box matmul N = NS*128 = 512)
NG = B_PER_CORE // NS
NK = 8  # s-chunks per sample (1024 / 128)
C = 128
S = 1024

# Engine knobs (TTR is broken on this HW path -> ss/zr avoid it):
#  U_ENG:      'pool' per-sample gpsimd broadcast TT | 'dve'
#  OUT_POOL_N: how many of the NS=4 samples per group compute the out
#              scale on gpsimd (rest on DVE)
U_ENG = os.environ.get("U_ENG", "pool")
OUT_POOL_N = int(os.environ.get("OUT_POOL_N", "2"))
OUT_BF16 = os.environ.get("OUT_BF16", "0") == "1"


def _consts():
    t32 = (np.abs(np.subtract.outer(np.arange(32), np.arange(32))) <= 1).astype(
        np.float32
    )
    a4 = (np.abs(np.subtract.outer(np.arange(4), np.arange(4))) <= 1).astype(
        np.float32
    )
    e30 = np.zeros((4, 4), np.float32)
    e30[3, 0] = 1.0
    e03 = np.zeros((4, 4), np.float32)
    e03[0, 3] = 1.0
    bd = np.kron(a4, t32) / 9.0
    bp = np.kron(e30, t32) / 9.0  # from chunk k-1
    bn = np.kron(e03, t32) / 9.0  # from chunk k+1
    wbox = np.stack([bd, bp, bn]).astype(ml_dtypes.bfloat16)
    return wbox


def build_nc():
    nc = bacc.Bacc()
    # p layout: [g, s128, k, b, c'] bf16  (host pre-transposed, c' = e*64+f;
    # chunk-major within each group of NS samples -> one 1MB DMA per group
    # with 8KB/partition contiguous lines)
    p_d = nc.declare_dram_parameter("p", [NG, 128, NK, NS, 128], BF16,
                                    isOutput=False)
    out_d = nc.declare_dram_parameter(
        "out", [B_PER_CORE, 2, NK, 128, 64], F32, isOutput=True
    )
    wbox_d = nc.declare_dram_parameter("wbox", [3, 128, 128], BF16,
                                       isOutput=False)

    with tile.TileContext(nc) as tc:
        with (
            tc.tile_pool(name="consts", bufs=1) as cpool,
            tc.tile_pool(name="pin", bufs=NG) as pinpool,
            tc.tile_pool(name="upool", bufs=3) as upool,
            tc.tile_pool(name="scr", bufs=3) as scrpool,
            tc.tile_pool(name="evac", bufs=6) as epool,
            tc.tile_pool(name="outp", bufs=2 * NS) as outpool,
            tc.tile_pool(name="stats", bufs=4 * NS) as stats,
            tc.tile_pool(name="psB", bufs=3, space="PSUM") as psB,
            tc.tile_pool(name="psW", bufs=1, space="PSUM") as psW,
        ):
            wbox = cpool.tile([128, 3, 128], BF16, tag="wbox")
            nc.sync.dma_start(wbox[:], wbox_d[:].transpose([1, 0, 2]))
            bd, bp, bn = wbox[:, 0, :], wbox[:, 1, :], wbox[:, 2, :]

            # startup observer: make PE's vector clock see the const-DMA
            # queue sem so steady-state matmuls only wait on u producers.
            scr1 = psW.tile([128, 1], F32, tag="warm")
            nc.tensor.matmul(scr1[:], bd, wbox[:, 0, 0:1], start=True, stop=True)

            # all input DMAs up front (64KB/partition total; all resident)
            pins = []
            for g in range(NG):
                pg = pinpool.tile([128, NK, NS, 128], BF16, tag="pin",
                                  name=f"pin_{g}")
                nc.sync.dma_start(pg[:], p_d[g])
                pins.append(pg)

            for g in range(NG):
                pg = pins[g]
                # u for whole group, chunk-major: ubig[:, k] = [128, NS, 128]
                # contiguous (matmul rhs); per-sample slice [:, :, b, :] is a
                # legal strided AP for the elementwise producers/consumers.
                ubig = upool.tile([128, NK, NS, 128], BF16, tag="u",
                                  name=f"u_{g}")
                # ---- phase A: norms + u ----
                sqb = scrpool.tile([128, NK, NS, 128], BF16, tag="sqb")
                nc.scalar.activation(sqb[:], pg[:], AF.Square)
                ssg = stats.tile([128, NK, NS], F32, tag="ss")
                nc.vector.tensor_reduce(ssg[:], sqb[:], axis=AX.X, op=ALU.add)
                nrmg = stats.tile([128, NK, NS], F32, tag="nrm")
                nc.scalar.sqrt(nrmg[:], ssg[:])
                rinvg = stats.tile([128, NK, NS], F32, tag="rinv",
                                   name=f"rinv_{g}")
                nc.vector.reciprocal(rinvg[:], nrmg[:])
                for b in range(NS):
                    rb = (rinvg[:, :, b].unsqueeze(2)
                          .broadcast_to([128, NK, 128]))
                    ueng = nc.gpsimd if U_ENG == "pool" else nc.vector
                    ueng.tensor_tensor(
                        ubig[:, :, b, :], pg[:, :, b, :], rb, op=ALU.mult
                    )
                zkg = stats.tile([128, NK, NS], F32, tag="zk",
                                 name=f"zk_{g}")

                # ---- phase B: box matmuls (chunk pairs) + evac + zr ----
                w2g = scrpool.tile([128, NK, NS, 128], BF16, tag="w2",
                                   name=f"w2_{g}")
                for kp in range(NK // 2):
                    k0 = 2 * kp
                    box = psB.tile([128, 2, NS, 128], F32, tag="box")
                    for dk in range(2):
                        k = k0 + dk
                        mms = [(bd, k)]
                        if k > 0:
                            mms.append((bp, k - 1))
                        if k < NK - 1:
                            mms.append((bn, k + 1))
                        for i, (w, j) in enumerate(mms):
                            nc.tensor.matmul(
                                box[:, dk],
                                w,
                                ubig[:, j],
                                start=(i == 0),
                                stop=(i == len(mms) - 1),
                            )
                    ev = epool.tile([128, 2, NS, 128], BF16, tag="ev")
                    nc.scalar.activation(ev[:], box[:], AF.Copy)
                    nc.vector.tensor_tensor(
                        w2g[:, k0 : k0 + 2], pg[:, k0 : k0 + 2], ev[:],
                        op=ALU.mult,
                    )
                nc.vector.tensor_reduce(
                    zkg[:], w2g[:], axis=AX.X, op=ALU.add
                )

                # ---- phase C: out scale + DMA ----
                for b in range(NS):
                    odt = BF16 if OUT_BF16 else F32
                    ot = outpool.tile([128, NK, 128], odt, tag="ot",
                                      name=f"ot_{g}_{b}")
                    zb = (zkg[:, :, b].unsqueeze(2)
                          .broadcast_to([128, NK, 128]))
                    oeng = nc.gpsimd if b < OUT_POOL_N else nc.vector
                    oeng.tensor_tensor(
                        ot[:], ubig[:, :, b, :], zb, op=ALU.mult
                    )
                    for e in range(2):
                        dst = out_d[g * NS + b, e].transpose([1, 0, 2])
                        src = ot[:, :, e * 64 : e * 64 + 64]
                        if OUT_BF16:
                            nc.gpsimd.dma_start(dst, src)
                        else:
                            nc.sync.dma_start(dst, src)

    nc.compile()
    return nc


def _prep_input(p_vector: np.ndarray) -> np.ndarray:
    """[256, 128, 32, 32] f32 -> [8, NG, 128, NK, NS, 128] bf16, layout
    [core, g, s128, k, b, c'] with c' = e*64+f (c = f*2+e)."""
    p = np.ascontiguousarray(p_vector, dtype=np.float32)
    arr = p.reshape(8, NG, NS, 64, 2, NK, 128)  # core,g,b,f,e,k,s128
    arr = arr.transpose(0, 1, 6, 5, 2, 4, 3)  # core,g,s128,k,b,e,f
    arr = np.ascontiguousarray(arr, dtype=ml_dtypes.bfloat16)
    return arr.reshape(8, NG, 128, NK, NS, 128)


_CACHE = {}


def kernel(p_vector: np.ndarray) -> np.ndarray:
    assert p_vector.shape == (256, 128, 32, 32)
    shards = _prep_input(p_vector)
    wbox = _consts()
    nc = build_nc()
    in_maps = [{"p": shards[i], "wbox": wbox} for i in range(8)]
    res = run_bass_kernel_spmd(nc, in_maps, core_ids=list(range(8)))
    outs = [r["out"].reshape(B_PER_CORE, 2048, 64) for r in res.results]
    return np.concatenate(outs, axis=0)


if __name__ == "__main__":
    x = np.random.randn(256, 128, 32, 32).astype(np.float32)
    y = kernel(x)
    print(y.shape, y.dtype)


# revision 26
# speedup vs baseline: 1.2214x; 1.2214x over previous
"""Trainium2 Bass kernel for nn_BCIM_45861660787130 (pooling / box-filter sim).

Math per sample (C=128 channels, 32x32 spatial = S=1024 pixels):
  unit = p / ||p||_C
  wmean = 3x3 zero-padded box mean of unit (per channel)
  sim = <unit, wmean>_C          # per pixel
  out = p * sim, then channel deinterleave c=(f*2+e) -> [e*S + s, f]

Strategy (data-parallel over batch, 32 samples/core):
  - HOST pre-pass: transpose each sample to pixel-major [s128, k, c'] with
    channels permuted to c' = e*64+f, and cast to bf16.  The device then
    needs NO transposes and NO channel shuffle; input DMA is halved and has
    2KB/partition contiguous lines.
  - Per (sample b, chunk k of 128 pixels):
      ss[s]  = sum_c pin^2           (DVE tensor_tensor_reduce)
      nrm    = sqrt(ss)  (ACT),  rinv = 1/nrm  (DVE)
      u      = pin * rinv            (DVE tensor_scalar, bf16 4x)
  - Box filter = block-tridiagonal matmuls on PE (bf16, N=512, groups of 4
    samples):  box_k = Bd^T u_k + Bp^T u_{k-1} + Bn^T u_{k+1}  (PSUM acc)
  - evac = box PSUM -> SBUF bf16 (ACT Copy, batched FD=512)
  - zr[s] = sum_c pin*evac = sim*nrm (DVE TTR);  out = u*zr = p*sim exactly
  - DMA out per (sample, e): [128, 8, 64] -> DRAM [8, 128, 64].
"""

import os
import sys

sys.path.insert(0, "/opt/trn_rl_repo")

import numpy as np
import ml_dtypes

from concourse import bacc, bass, mybir, tile
from concourse.bass_utils import run_bass_kernel_spmd

F32 = mybir.dt.float32
BF16 = mybir.dt.bfloat16
AF = mybir.ActivationFunctionType
ALU = mybir.AluOpType
AX = mybir.AxisListType

B_PER_CORE = 32  # samples per core
NS = 4  # samples per group (box matmul N = NS*128 = 512)
NG = B_PER_CORE // NS
NK = 8  # s-chunks per sample (1024 / 128)
C = 128
S = 1024

# Engine knobs (TTR is broken on this HW path -> ss/zr avoid it):
#  U_ENG:      'pool' per-sample gpsimd broadcast TT | 'dve'
#  OUT_POOL_N: how many of the NS=4 samples per group compute the out
#              scale on gpsimd (rest on DVE)
U_ENG = os.environ.get("U_ENG", "pool")
OUT_POOL_N = int(os.environ.get("OUT_POOL_N", "2"))
OUT_BF16 = os.environ.get("OUT_BF16", "0") == "1"


def _consts():
    t32 = (np.abs(np.subtract.outer(np.arange(32), np.arange(32))) <= 1).astype(
        np.float32
    )
    a4 = (np.abs(np.subtract.outer(np.arange(4), np.arange(4))) <= 1).astype(
        np.float32
    )
    e30 = np.zeros((4, 4), np.float32)
    e30[3, 0] = 1.0
    e03 = np.zeros((4, 4), np.float32)
    e03[0, 3] = 1.0
    bd = np.kron(a4, t32) / 9.0
    bp = np.kron(e30, t32) / 9.0  # from chunk k-1
    bn = np.kron(e03, t32) / 9.0  # from chunk k+1
    wbox = np.stack([bd, bp, bn]).astype(ml_dtypes.bfloat16)
    return wbox


def build_nc():
    nc = bacc.Bacc()
    # p layout: [g, s128, k, b, c'] bf16  (host pre-transposed, c' = e*64+f;
    # chunk-major within each group of NS samples -> one 1MB DMA per group
    # with 8KB/partition contiguous lines)
    p_d = nc.declare_dram_parameter("p", [NG, 128, NK, NS, 128], BF16,
                                    isOutput=False)
    out_d = nc.declare_dram_parameter(
        "out", [B_PER_CORE, 2, NK, 128, 64], F32, isOutput=True
    )
    wbox_d = nc.declare_dram_parameter("wbox", [3, 128, 128], BF16,
                                       isOutput=False)

    with tile.TileContext(nc) as tc:
        with (
            tc.tile_pool(name="consts", bufs=1) as cpool,
            tc.tile_pool(name="pin", bufs=NG) as pinpool,
            tc.tile_pool(name="upool", bufs=2) as upool,
            tc.tile_pool(name="scr", bufs=3) as scrpool,
            tc.tile_pool(name="evac", bufs=4) as epool,
            tc.tile_pool(name="outp", bufs=2 * NS) as outpool,
            tc.tile_pool(name="stats", bufs=4 * NS) as stats,
            tc.tile_pool(name="psB", bufs=3, space="PSUM") as psB,
            tc.tile_pool(name="psW", bufs=1, space="PSUM") as psW,
        ):
            wbox = cpool.tile([128, 3, 128], BF16, tag="wbox")
            nc.sync.dma_start(wbox[:], wbox_d[:].transpose([1, 0, 2]))
            bd, bp, bn = wbox[:, 0, :], wbox[:, 1, :], wbox[:, 2, :]

            # startup observer: make PE's vector clock see the const-DMA
            # queue sem so steady-state matmuls only wait on u producers.
            scr1 = psW.tile([128, 1], F32, tag="warm")
            nc.tensor.matmul(scr1[:], bd, wbox[:, 0, 0:1], start=True, stop=True)

            # all input DMAs up front (64KB/partition total; all resident)
            pins = []
            for g in range(NG):
                pg = pinpool.tile([128, NK, NS, 128], BF16, tag="pin",
                                  name=f"pin_{g}")
                nc.sync.dma_start(pg[:], p_d[g])
                pins.append(pg)

            for g in range(NG):
                pg = pins[g]
                # u for whole group, chunk-major: ubig[:, k] = [128, NS, 128]
                # contiguous (matmul rhs); per-sample slice [:, :, b, :] is a
                # legal strided AP for the elementwise producers/consumers.
                ubig = upool.tile([128, NK, NS, 128], BF16, tag="u",
                                  name=f"u_{g}")
                # ---- phase A: norms + u ----
                sqb = scrpool.tile([128, NK, NS, 128], BF16, tag="sqb")
                nc.scalar.activation(sqb[:], pg[:], AF.Square)
                ssg = stats.tile([128, NK, NS], F32, tag="ss")
                nc.vector.tensor_reduce(ssg[:], sqb[:], axis=AX.X, op=ALU.add)
                nrmg = stats.tile([128, NK, NS], F32, tag="nrm")
                nc.scalar.sqrt(nrmg[:], ssg[:])
                rinvg = stats.tile([128, NK, NS], F32, tag="rinv",
                                   name=f"rinv_{g}")
                nc.vector.reciprocal(rinvg[:], nrmg[:])
                for b in range(NS):
                    rb = (rinvg[:, :, b].unsqueeze(2)
                          .broadcast_to([128, NK, 128]))
                    ueng = nc.gpsimd if U_ENG == "pool" else nc.vector
                    ueng.tensor_tensor(
                        ubig[:, :, b, :], pg[:, :, b, :], rb, op=ALU.mult
                    )
                zkg = stats.tile([128, NK, NS], F32, tag="zk",
                                 name=f"zk_{g}")

                # ---- phase B: box matmuls (chunk pairs) + evac + zr ----
                w2g = scrpool.tile([128, NK, NS, 128], BF16, tag="w2",
                                   name=f"w2_{g}")
                for kp in range(NK // 2):
                    k0 = 2 * kp
                    box = psB.tile([128, 2, NS, 128], F32, tag="box")
                    for dk in range(2):
                        k = k0 + dk
                        mms = [(bd, k)]
                        if k > 0:
                            mms.append((bp, k - 1))
                        if k < NK - 1:
                            mms.append((bn, k + 1))
                        for i, (w, j) in enumerate(mms):
                            nc.tensor.matmul(
                                box[:, dk],
                                w,
                                ubig[:, j],
                                start=(i == 0),
                                stop=(i == len(mms) - 1),
                            )
                    ev = epool.tile([128, 2, NS, 128], BF16, tag="ev")
                    nc.scalar.activation(ev[:], box[:], AF.Copy)
                    nc.vector.tensor_tensor(
                        w2g[:, k0 : k0 + 2], pg[:, k0 : k0 + 2], ev[:],
                        op=ALU.mult,
                    )
                nc.vector.tensor_reduce(
                    zkg[:], w2g[:], axis=AX.X, op=ALU.add
                )

                # ---- phase C: out scale + DMA ----
                for b in range(NS):
                    odt = BF16 if OUT_BF16 else F32
                    ot = outpool.tile([128, NK, 128], odt, tag="ot",
                                      name=f"ot_{g}_{b}")
                    zb = (zkg[:, :, b].unsqueeze(2)
                          .broadcast_to([128, NK, 128]))
                    oeng = nc.gpsimd if b < OUT_POOL_N else nc.vector
                    oeng.tensor_tensor(
                        ot[:], ubig[:, :, b, :], zb, op=ALU.mult
                    )
                    for e in range(2):
                        dst = out_d[g * NS + b, e].transpose([1, 0, 2])
                        src = ot[:, :, e * 64 : e * 64 + 64]
                        if OUT_BF16:
                            nc.gpsimd.dma_start(dst, src)
                        else:
                            nc.sync.dma_start(dst, src)

    nc.compile()
    return nc


def _prep_input(p_vector: np.ndarray) -> np.ndarray:
    """[256, 128, 32, 32] f32 -> [8, NG, 128, NK, NS, 128] bf16, layout
    [core, g, s128, k, b, c'] with c' = e*64+f (c = f*2+e)."""
    p = np.ascontiguousarray(p_vector, dtype=np.float32)
    arr = p.reshape(8, NG, NS, 64, 2, NK, 128)  # core,g,b,f,e,k,s128
    arr = arr.transpose(0, 1, 6, 5, 2, 4, 3)  # core,g,s128,k,b,e,f
    arr = np.ascontiguousarray(arr, dtype=ml_dtypes.bfloat16)
    return arr.reshape(8, NG, 128, NK, NS, 128)


_CACHE = {}


def kernel(p_vector: np.ndarray) -> np.ndarray:
    assert p_vector.shape == (256, 128, 32, 32)
    shards = _prep_input(p_vector)
    wbox = _consts()
    nc = build_nc()
    in_maps = [{"p": shards[i], "wbox": wbox} for i in range(8)]
    res = run_bass_kernel_spmd(nc, in_maps, core_ids=list(range(8)))
    outs = [r["out"].reshape(B_PER_CORE, 2048, 64) for r in res.results]
    return np.concatenate(outs, axis=0)


if __name__ == "__main__":
    x = np.random.randn(256, 128, 32, 32).astype(np.float32)
    y = kernel(x)
    print(y.shape, y.dtype)
